# revision 10
# baseline (speedup 1.0000x reference)
"""GAT link-prediction kernel for 8 Trainium2 NeuronCores (Bass/Tile).

Sharding: nodes are relabeled (degree-dealt round-robin) so each core owns a
contiguous range of 6272 node ids and ~1/8 of the edges, grouped by
destination node -> all segment-softmax/scatter-sum work is core-local.
Each layer every core recomputes the full node table [alpha_l | hr] (bf16)
from an AllGather'ed activation, writes it to DRAM, and fetches per-edge rows
with the gpsimd dma_gather custom op. dma_gather uses int16 indices, so the
table is split in low/high halves and each destination node's in-edges are
processed in two passes, each tiled by that pass's in-degree (pad slots point
at a row whose alpha_l is -3000, which turns into an exact softmax zero).
Per-node softmax sums accumulate in SBUF (the high pass in its own node order,
merged back through a DRAM round-trip gather); BatchNorm stats use a small
AllReduce; the decoder runs on a 1/8 shard of the link-prediction edges.
"""

import sys
import numpy as np

if "/opt/trn_rl_repo" not in sys.path:
    sys.path.insert(0, "/opt/trn_rl_repo")

import ml_dtypes

BF16 = ml_dtypes.bfloat16

N = 50000
E = 800000
EL = 131072
DF, DH, DZ = 128, 96, 96
EPS_BN = 1e-5
NCORES = 8
NPC = 6272
NPAD = NPC * NCORES          # 50176
NT = NPC // 128              # 49
HALF = NPAD // 2             # 25088
TROWS = HALF + 128           # rows per half table (pad row at local id HALF)
ROW = 256                    # bf16 row: [alpha_l 0:96 | pad | hr 128:224 | pad]
NEG = -3000.0
ELC = EL // NCORES           # 16384
DEC_CH = 2048

_CACHE = {}


def _wrap16(vals, nidx):
    a = np.zeros((16, nidx // 16), np.int16)
    idx = np.arange(nidx)
    a[idx % 16, idx // 16] = np.asarray(vals, np.int64).astype(np.int16)
    return np.tile(a, (8, 1))


def _prep(edge_index, edges):
    src = np.asarray(edge_index[0], np.int64)
    dst = np.asarray(edge_index[1], np.int64)
    deg = np.bincount(dst, minlength=N)
    order = np.argsort(-deg, kind="stable")
    lane = np.arange(N) % (2 * NCORES)
    lane = np.where(lane < NCORES, lane, 2 * NCORES - 1 - lane)
    core_of = np.empty(N, np.int64)
    core_of[order] = lane
    groups = [order[core_of[order] == c] for c in range(NCORES)]

    def ids_from(groups_):
        old_of_new = np.full(NPAD, -1, np.int64)
        for c, g in enumerate(groups_):
            old_of_new[c * NPC:c * NPC + len(g)] = g
        new_of_old = np.full(N, -1, np.int64)
        m = old_of_new >= 0
        new_of_old[old_of_new[m]] = np.nonzero(m)[0]
        return new_of_old, old_of_new

    new_of_old, _ = ids_from(groups)
    lo_cnt = np.bincount(new_of_old[dst[new_of_old[src] < HALF]], minlength=NPAD)
    groups = [g[np.argsort(lo_cnt[new_of_old[g]], kind="stable")] for g in groups]
    new_of_old, old_of_new = ids_from(groups)
    nsrc, ndst = new_of_old[src], new_of_old[dst]

    cores = []
    dmax = 1
    for c in range(NCORES):
        base = c * NPC
        m = (ndst >= base) & (ndst < base + NPC)
        es, ed = nsrc[m], ndst[m] - base
        passes = []
        for hi in (0, 1):
            pm = (es >= HALF) if hi else (es < HALF)
            ps_ = es[pm] - (HALF if hi else 0)
            pd = ed[pm]
            pdeg = np.bincount(pd, minlength=NPC)
            nodes = np.arange(NPC) if not hi else np.argsort(pdeg, kind="stable")
            o2 = np.argsort(pd, kind="stable")
            ps_sorted = ps_[o2]
            starts = np.zeros(NPC + 1, np.int64)
            np.cumsum(pdeg, out=starts[1:])
            dbars, idxs = [], []
            for t in range(NT):
                tn = nodes[t * 128:(t + 1) * 128]
                db = int(pdeg[tn].max())
                dbars.append(db)
                dmax = max(dmax, db)
                if db == 0:
                    idxs.append(None)
                    continue
                idx = np.full(128 * db, HALF, np.int64)
                for p, nloc in enumerate(tn):
                    d0 = int(pdeg[nloc])
                    if d0:
                        idx[np.arange(d0) * 128 + p] = \
                            ps_sorted[starts[nloc]:starts[nloc] + d0]
                idxs.append(idx)
            passes.append(dict(nodes=nodes, dbars=dbars, idxs=idxs))
        cores.append(passes)

    tiles_meta = []
    for p in range(2):
        meta, off = [], 0
        for t in range(NT):
            db = max(cores[c][p]["dbars"][t] for c in range(NCORES))
            meta.append((db, off))
            off += 8 * db
        tiles_meta.append((meta, off))

    idx_main = []
    for p in range(2):
        meta, total = tiles_meta[p]
        arrs = []
        for c in range(NCORES):
            cols = np.zeros((128, max(total, 16)), np.int16)
            for t in range(NT):
                db, off = meta[t]
                if db == 0:
                    continue
                idx = np.full(128 * db, HALF, np.int64)
                own = cores[c][p]["idxs"][t]
                if own is not None:
                    idx[:len(own)] = own
                cols[:, off:off + 8 * db] = _wrap16(idx, 128 * db)
            arrs.append(cols)
        idx_main.append(arrs)

    ei = new_of_old[np.asarray(edges)[:, 0]]
    ej = new_of_old[np.asarray(edges)[:, 1]]
    return dict(cores=cores, tiles_meta=tiles_meta, idx_main=idx_main,
                new_of_old=new_of_old, old_of_new=old_of_new, ei=ei, ej=ej,
                dmax=dmax)


def _build(host):
    import concourse.bacc as bacc
    import concourse.mybir as mb
    from concourse.tile import TileContext
    from concourse import masks

    f32, bf16, i16 = mb.dt.float32, mb.dt.bfloat16, mb.dt.int16
    AF = mb.ActivationFunctionType
    OP = mb.AluOpType
    DMAX = host["dmax"]

    nc = bacc.Bacc(None, target_bir_lowering=False)
    D = nc.dram_tensor

    featT = D("featT", [DF, NPAD], bf16, kind="ExternalInput")
    wcat = [D(f"wcat{l}", [DF if l == 0 else DH, ROW], bf16, kind="ExternalInput")
            for l in range(3)]
    arv = [D(f"arv{l}", [1, DH], f32, kind="ExternalInput") for l in range(3)]
    gamv = [D(f"gam{l}", [1, DH], f32, kind="ExternalInput") for l in range(3)]
    betv = [D(f"bet{l}", [1, DH], f32, kind="ExternalInput") for l in range(3)]
    idxm = [D(f"idxm{p}", [128, max(host["tiles_meta"][p][1], 16)], i16,
              kind="ExternalInput") for p in range(2)]
    idxhi = D("idxhi", [128, NPC // 16], i16, kind="ExternalInput")
    idxar = [D(f"idxar{k}", [128, NPC // 16], i16, kind="ExternalInput")
             for k in range(4)]
    idxd = [D(f"idxd{k}", [128, ELC // 16], i16, kind="ExternalInput")
            for k in range(4)]
    tfv = D("tf", [1, ELC], f32, kind="ExternalInput")
    tcfv = D("tcf", [1, ELC], f32, kind="ExternalInput")
    w1l = D("w1l", [DZ + 1, DH], bf16, kind="ExternalInput")
    w2v = D("w2", [DH, 1], bf16, kind="ExternalInput")

    z_out = D("z_out", [NPC, DZ], f32, kind="ExternalOutput")
    pf_out = D("pf", [1, ELC], f32, kind="ExternalOutput")
    pcf_out = D("pcf", [1, ELC], f32, kind="ExternalOutput")

    tab = [D("tab_lo", [TROWS, ROW], bf16), D("tab_hi", [TROWS, ROW], bf16)]
    ard = [D("ar_lo", [TROWS, 128], bf16), D("ar_hi", [TROWS, 128], bf16)]
    acchi_dram = D("acchi_dram", [NPC, 192], f32)
    zjk_dram = D("zjk_dram", [128, NT * DZ], f32)
    zgt = [D("zg_lo", [TROWS, ROW], bf16), D("zg_hi", [TROWS, ROW], bf16)]
    ag_in = D("ag_in", [DH, NPC], bf16)
    ag_out = D("ag_out", [NCORES * DH, NPC], bf16)
    zag_in = D("zag_in", [NPC, ROW], bf16)
    zag_out = D("zag_out", [NPAD, ROW], bf16)
    st_in = D("st_in", [1, 192], f32)
    st_out = D("st_out", [1, 192], f32)

    groups = [list(range(NCORES))]
    tiles_meta = host["tiles_meta"]

    with TileContext(nc) as tc:
        with tc.tile_pool(name="per", bufs=1) as per, \
             tc.tile_pool(name="work", bufs=2) as work, \
             tc.tile_pool(name="wbig", bufs=1) as wb, \
             tc.tile_pool(name="ps", bufs=2, space="PSUM") as ps, \
             tc.tile_pool(name="ps1", bufs=1, space="PSUM") as ps1, \
             tc.tile_pool(name="accp", bufs=1) as accp:

            ident = per.tile([128, 128], bf16)
            masks.make_identity(nc, ident[:, :])
            ones_col = per.tile([128, 1], f32)
            nc.vector.memset(ones_col[:, :], 1.0)
            padrow = per.tile([1, ROW], bf16)
            nc.vector.memset(padrow[:, :], 0.0)
            nc.vector.memset(padrow[:, 0:DH], NEG)
            zrow = per.tile([1, ROW], bf16)
            nc.vector.memset(zrow[:, :], 0.0)
            for h in range(2):
                nc.sync.dma_start(out=tab[h][HALF:HALF + 1, :], in_=padrow[:, :])
                nc.sync.dma_start(out=zgt[h][HALF:HALF + 1, :], in_=zrow[:, :])
                nc.sync.dma_start(out=ard[h][HALF:HALF + 1, :],
                                  in_=zrow[:, 0:128])

            ihit = per.tile([128, NPC // 16], i16)
            nc.sync.dma_start(out=ihit[:, :], in_=idxhi[:, :])
            iart = [per.tile([128, NPC // 16], i16, tag=f"iart{k}", name=f"iart{k}")
                    for k in range(4)]
            for k in range(4):
                nc.sync.dma_start(out=iart[k][:, :], in_=idxar[k][:, :])

            with tc.tile_pool(name="gbig", bufs=2) as gp, \
                 tc.tile_pool(name="arsp", bufs=1) as arsp:
                imt = [per.tile([128, max(tiles_meta[p][1], 16)], i16,
                                tag=f"imt{p}", name=f"imt{p}") for p in range(2)]
                for p in range(2):
                    nc.sync.dma_start(out=imt[p][:, :], in_=idxm[p][:, :])

                for l in range(3):
                    cin = DF if l == 0 else DH
                    wct = per.tile([128, ROW], bf16, tag="wct")
                    nc.sync.dma_start(out=wct[0:cin, :], in_=wcat[l][:, :])
                    art = work.tile([1, DH], f32, tag="art")
                    nc.sync.dma_start(out=art[:, :], in_=arv[l][:, :])
                    arf = work.tile([128, DH], f32, tag="arf")
                    nc.vector.tensor_copy(arf[0:1, :], art[:, :])
                    nc.gpsimd.partition_broadcast(arf[:, :], arf[0:1, :])
                    ar_rep = work.tile([128, DH], bf16, tag="ar_rep")
                    nc.vector.tensor_copy(ar_rep[:, :], arf[:, :])

                    # ---------- dense phase ----------
                    for tb in range(NPAD // 512):
                        rows4 = work.tile([128, 4, ROW], bf16, tag="rows4")
                        arw4 = work.tile([128, 4, 128], bf16, tag="arw4")
                        for q in range(4):
                            t = tb * 4 + q
                            xw = work.tile([128, 128], bf16, tag="xw")
                            if l == 0:
                                nc.sync.dma_start(
                                    out=xw[0:cin, :],
                                    in_=featT[:, t * 128:(t + 1) * 128])
                            else:
                                cb, o = t // NT, (t % NT) * 128
                                nc.sync.dma_start(
                                    out=xw[0:cin, :],
                                    in_=ag_out[cb * DH:(cb + 1) * DH, o:o + 128])
                            pst = ps.tile([128, ROW], f32, tag="pst")
                            nc.tensor.matmul(pst[:, :], xw[0:cin, :],
                                             wct[0:cin, :], start=True, stop=True)
                            nc.scalar.copy(rows4[:, q, 0:128], pst[:, 0:128])
                            nc.vector.tensor_copy(rows4[:, q, 128:ROW],
                                                  pst[:, 128:ROW])
                            nc.vector.tensor_mul(arw4[:, q, 0:DH],
                                                 rows4[:, q, 128:128 + DH],
                                                 ar_rep[:, :])
                            nc.vector.memset(arw4[:, q, DH:128], 0.0)
                        nb = tb * 512
                        h = 0 if nb < HALF else 1
                        r0 = nb - h * HALF
                        nc.sync.dma_start(
                            out=tab[h][r0:r0 + 512, :].rearrange(
                                "(q p) r -> p q r", p=128),
                            in_=rows4[:, :, :])
                        nc.sync.dma_start(
                            out=ard[h][r0:r0 + 512, :].rearrange(
                                "(q p) r -> p q r", p=128),
                            in_=arw4[:, :, :])

                    # ---------- edge phase: hi pass first, then lo ----------
                    for p in (1, 0):
                        ars = accp.tile([128, NT, 128], bf16, tag="ars")
                        art2 = arsp.tile([128, NT, 128], bf16, tag="arstmp")
                        nc.gpsimd.dma_gather(
                            out_ap=ars[:, :, :], in_ap=ard[0][:, :],
                            idxs_ap=iart[2 * p][:, :], num_idxs=NPC,
                            num_idxs_reg=NPC, elem_size=128, single_packet=False)
                        nc.gpsimd.dma_gather(
                            out_ap=art2[:, :, :], in_ap=ard[1][:, :],
                            idxs_ap=iart[2 * p + 1][:, :], num_idxs=NPC,
                            num_idxs_reg=NPC, elem_size=128, single_packet=False)
                        nc.vector.tensor_add(ars[:, :, :], ars[:, :, :],
                                             art2[:, :, :])
                        acc = accp.tile([128, NT * 192], f32, tag="acc")
                        meta, _tot = tiles_meta[p]
                        for t in range(NT):
                            dbar, off = meta[t]
                            if dbar == 0:
                                nc.vector.memset(acc[:, t * 192:(t + 1) * 192], 0.0)
                                continue
                            nidx = 128 * dbar
                            g = gp.tile([128, DMAX, ROW], bf16, tag="g")
                            nc.gpsimd.dma_gather(
                                out_ap=g[:, 0:dbar, :], in_ap=tab[p][:, :],
                                idxs_ap=imt[p][:, off:off + 8 * dbar],
                                num_idxs=nidx, num_idxs_reg=nidx,
                                elem_size=ROW, single_packet=False)
                            ga = g[:, 0:dbar, 0:DH]
                            gh = g[:, 0:dbar, 128:128 + DH]
                            arb = ars[:, t:t + 1, 0:DH].broadcast_to(
                                [128, dbar, DH])
                            nc.vector.tensor_add(ga, ga, arb)
                            nc.scalar.activation(ga, ga, AF.Lrelu, 0.0, 1.0, 0.01)
                            nc.scalar.activation(ga, ga, AF.Exp)
                            nc.vector.tensor_mul(gh, gh, ga)
                            nc.vector.tensor_reduce(
                                acc[:, t * 192:t * 192 + 96],
                                g[:, 0:dbar, 0:DH].rearrange("p b c -> p c b"),
                                axis=mb.AxisListType.X, op=OP.add)
                            nc.vector.tensor_reduce(
                                acc[:, t * 192 + 96:(t + 1) * 192],
                                g[:, 0:dbar, 128:128 + DH].rearrange(
                                    "p b c -> p c b"),
                                axis=mb.AxisListType.X, op=OP.add)
                        if p == 1:
                            nc.sync.dma_start(
                                out=acchi_dram[:, :].rearrange(
                                    "(t p) r -> p t r", p=128),
                                in_=acc[:, :].rearrange("p (t r) -> p t r", r=192))
                    # merge hi pass back (two half-chunks to bound sbuf)
                    for hh in range(2):
                        t0 = hh * 25
                        tn = 25 if hh == 0 else NT - 25
                        accg = wb.tile([128, 25, 192], f32, tag="wA")
                        nc.gpsimd.dma_gather(
                            out_ap=accg[:, 0:tn, :], in_ap=acchi_dram[:, :],
                            idxs_ap=ihit[:, t0 * 8:(t0 + tn) * 8],
                            num_idxs=tn * 128, num_idxs_reg=tn * 128,
                            elem_size=192, single_packet=False)
                        nc.vector.tensor_add(
                            acc[:, t0 * 192:(t0 + tn) * 192],
                            acc[:, t0 * 192:(t0 + tn) * 192],
                            accg[:, 0:tn, :].rearrange("p t r -> p (t r)"))

                    # out = num/(s+eps), overwriting the s slots of acc
                    a3 = acc[:, :].rearrange("p (t r) -> p t r", r=192)
                    rcp = wb.tile([128, NT, DH], f32, tag="wB")
                    nc.vector.tensor_scalar_add(rcp[:, :, :], a3[:, :, 0:96],
                                                1e-16)
                    nc.vector.reciprocal(rcp[:, :, :], rcp[:, :, :])
                    outp = a3[:, :, 0:96]
                    nc.vector.tensor_mul(outp, a3[:, :, 96:192], rcp[:, :, :])

                    # ---------- batchnorm + elu + jk ----------
                    sq = wb.tile([128, NT, DH], f32, tag="wC")
                    nc.scalar.activation(sq[:, :, :], outp, AF.Square)
                    pstat = ps1.tile([1, 192], f32, tag="pstat")
                    for t in range(NT):
                        rhs = work.tile([128, 192], f32, tag="bnrhs")
                        nc.vector.tensor_copy(rhs[:, 0:96], a3[:, t, 0:96])
                        nc.vector.tensor_copy(rhs[:, 96:192], sq[:, t, :])
                        nc.tensor.matmul(pstat[:, :], ones_col[:, :], rhs[:, :],
                                         start=(t == 0), stop=(t == NT - 1))
                    stat = work.tile([1, 192], f32, tag="stat")
                    nc.vector.tensor_copy(stat[:, :], pstat[:, :])
                    nc.sync.dma_start(out=st_in[:, :], in_=stat[:, :])
                    nc.gpsimd.collective_compute(
                        "AllReduce", OP.add, replica_groups=groups,
                        ins=[st_in[:, :].opt()], outs=[st_out[:, :].opt()])
                    stg = work.tile([1, 192], f32, tag="statg")
                    nc.sync.dma_start(out=stg[:, :], in_=st_out[:, :])
                    gmt = work.tile([1, DH], f32, tag="gmt")
                    btt = work.tile([1, DH], f32, tag="btt")
                    nc.sync.dma_start(out=gmt[:, :], in_=gamv[l][:, :])
                    nc.sync.dma_start(out=btt[:, :], in_=betv[l][:, :])
                    mu = work.tile([1, DH], f32, tag="mu")
                    var = work.tile([1, DH], f32, tag="var")
                    nc.vector.tensor_scalar_mul(mu[:, :], stg[:, 0:96], 1.0 / N)
                    nc.vector.tensor_scalar_mul(var[:, :], stg[:, 96:192], 1.0 / N)
                    msq = work.tile([1, DH], f32, tag="msq")
                    nc.vector.tensor_mul(msq[:, :], mu[:, :], mu[:, :])
                    nc.vector.tensor_sub(var[:, :], var[:, :], msq[:, :])
                    nc.vector.tensor_scalar_add(var[:, :], var[:, :], EPS_BN)
                    rst = work.tile([1, DH], f32, tag="rst")
                    nc.vector.reciprocal(rst[:, :], var[:, :])
                    nc.scalar.activation(rst[:, :], rst[:, :], AF.Sqrt)
                    scl = work.tile([1, DH], f32, tag="scl")
                    nc.vector.tensor_mul(scl[:, :], gmt[:, :], rst[:, :])
                    shf = work.tile([1, DH], f32, tag="shf")
                    nc.vector.tensor_mul(shf[:, :], mu[:, :], scl[:, :])
                    nc.vector.tensor_sub(shf[:, :], btt[:, :], shf[:, :])
                    srep = work.tile([128, DH], f32, tag="srep")
                    hrep = work.tile([128, DH], f32, tag="hrep")
                    nc.vector.tensor_copy(srep[0:1, :], scl[:, :])
                    nc.vector.tensor_copy(hrep[0:1, :], shf[:, :])
                    nc.gpsimd.partition_broadcast(srep[:, :], srep[0:1, :])
                    nc.gpsimd.partition_broadcast(hrep[:, :], hrep[0:1, :])
                    sb = srep[:, :].rearrange("p (o c) -> p o c", o=1) \
                        .broadcast_to([128, NT, DH])
                    hb = hrep[:, :].rearrange("p (o c) -> p o c", o=1) \
                        .broadcast_to([128, NT, DH])
                    nc.vector.tensor_mul(outp, outp, sb)
                    nc.vector.tensor_add(outp, outp, hb)
                    mny = wb.tile([128, NT, DH], f32, tag="wB")
                    nc.vector.tensor_scalar_min(mny[:, :, :], outp, 0.0)
                    nc.scalar.activation(mny[:, :, :], mny[:, :, :], AF.Exp)
                    pxy = wb.tile([128, NT, DH], f32, tag="wC")
                    nc.vector.tensor_scalar_max(pxy[:, :, :], outp, 0.0)
                    xn = mny
                    nc.vector.scalar_tensor_tensor(
                        xn[:, :, :], mny[:, :, :], -1.0, pxy[:, :, :],
                        OP.add, OP.add)
                    zjk = wb.tile([128, NT, DZ], f32, tag="wC")
                    if l == 0:
                        nc.vector.tensor_scalar_mul(
                            zjk[:, :, :], xn[:, :, :], float(host["wjk"][0]))
                    else:
                        nc.sync.dma_start(
                            out=zjk[:, :, :],
                            in_=zjk_dram[:, :].rearrange(
                                "p (t c) -> p t c", c=DZ))
                        nc.vector.scalar_tensor_tensor(
                            zjk[:, :, :], xn[:, :, :], float(host["wjk"][l]),
                            zjk[:, :, :], OP.mult, OP.add)
                    nc.sync.dma_start(
                        out=zjk_dram[:, :].rearrange("p (t c) -> p t c", c=DZ),
                        in_=zjk[:, :, :])
                    if l < 2:
                        xb = wb.tile([128, NT, DH], bf16, tag="wA")
                        nc.vector.tensor_copy(xb[:, :, :], xn[:, :, :])
                        xT = wb.tile([DH, NT * 128], bf16, tag="xT")
                        for t in range(NT):
                            pt = ps1.tile([DH, 128], bf16, tag="ptr")
                            nc.tensor.transpose(pt[:, :], xb[:, t, :],
                                                ident[:, :])
                            nc.scalar.copy(xT[:, t * 128:(t + 1) * 128],
                                           pt[:, :])
                        nc.sync.dma_start(out=ag_in[:, :], in_=xT[:, :])
                        nc.gpsimd.collective_compute(
                            "AllGather", OP.bypass, replica_groups=groups,
                            ins=[ag_in[:, :].opt()], outs=[ag_out[:, :].opt()])

                # ---------------- z outputs (inside gbig scope end) --------
                zjk = wb.tile([128, NT, DZ], f32, tag="wC")
                nc.sync.dma_start(
                    out=zjk[:, :, :],
                    in_=zjk_dram[:, :].rearrange("p (t c) -> p t c", c=DZ))
                nc.sync.dma_start(
                    out=z_out[:, :].rearrange("(t p) c -> p t c", p=128),
                    in_=zjk[:, :, :])
                for hh in range(2):
                    t0 = hh * 25
                    tn = 25 if hh == 0 else NT - 25
                    zb = wb.tile([128, 25, ROW], bf16, tag="wA", name="zb")
                    nc.vector.memset(zb[:, 0:tn, :], 0.0)
                    nc.vector.tensor_copy(zb[:, 0:tn, 0:DZ],
                                          zjk[:, t0:t0 + tn, :])
                    nc.sync.dma_start(
                        out=zag_in[t0 * 128:(t0 + tn) * 128, :].rearrange(
                            "(t p) r -> p t r", p=128),
                        in_=zb[:, 0:tn, :])
                nc.gpsimd.collective_compute(
                    "AllGather", OP.bypass, replica_groups=groups,
                    ins=[zag_in[:, :].opt()], outs=[zag_out[:, :].opt()])
                nc.sync.dma_start(out=zgt[0][0:HALF, :], in_=zag_out[0:HALF, :])
                nc.sync.dma_start(out=zgt[1][0:HALF, :],
                                  in_=zag_out[HALF:NPAD, :])

            # ---------------- decode ----------------
            with tc.tile_pool(name="dec", bufs=1) as dp:
                idt = [per.tile([128, ELC // 16], i16, tag=f"idt{k}", name=f"idt{k}")
                       for k in range(4)]
                for k in range(4):
                    nc.sync.dma_start(out=idt[k][:, :], in_=idxd[k][:, :])
                w1t = per.tile([DZ + 1, DH], bf16, tag="w1t")
                nc.sync.dma_start(out=w1t[:, :], in_=w1l[:, :])
                w2t = per.tile([DH, 1], bf16, tag="w2t")
                nc.sync.dma_start(out=w2t[:, :], in_=w2v[:, :])

                KD = DEC_CH // 128
                for ce in range(ELC // DEC_CH):
                    e0 = ce * DEC_CH
                    zi = dp.tile([128, KD, ROW], bf16, tag="zi")
                    zj = dp.tile([128, KD, ROW], bf16, tag="zj")
                    zt = dp.tile([128, KD, ROW], bf16, tag="zt")
                    for side, dstt in ((0, zi), (1, zj)):
                        for h in range(2):
                            tgt = dstt if h == 0 else zt
                            nc.gpsimd.dma_gather(
                                out_ap=tgt[:, :, :], in_ap=zgt[h][:, :],
                                idxs_ap=idt[2 * side + h][
                                    :, e0 // 16:(e0 + DEC_CH) // 16],
                                num_idxs=DEC_CH, num_idxs_reg=DEC_CH,
                                elem_size=ROW, single_packet=False)
                        nc.vector.tensor_add(dstt[:, :, :], dstt[:, :, :],
                                             zt[:, :, :])
                    hprod = dp.tile([128, KD, DH], bf16, tag="hprod")
                    nc.vector.tensor_mul(hprod[:, :, :], zi[:, :, 0:DH],
                                         zj[:, :, 0:DH])
                    hT = dp.tile([DZ + 1, DEC_CH], bf16, tag="hT")
                    for b in range(KD):
                        ptp = ps1.tile([DH, 128], bf16, tag="ptr")
                        nc.tensor.transpose(ptp[:, :], hprod[:, b, :],
                                            ident[:, :])
                        nc.scalar.copy(hT[0:DH, b * 128:(b + 1) * 128],
                                       ptp[:, :])
                    for tsrc, pout in ((tfv, pf_out), (tcfv, pcf_out)):
                        nc.gpsimd.dma_start(out=hT[DZ:DZ + 1, :],
                                            in_=tsrc[:, e0:e0 + DEC_CH])
                        orow = dp.tile([1, DEC_CH], f32, tag="orow")
                        for q in range(DEC_CH // 512):
                            ph = ps1.tile([DH, 512], f32, tag="ph")
                            nc.tensor.matmul(ph[:, :], w1t[:, :],
                                             hT[:, q * 512:(q + 1) * 512],
                                             start=True, stop=True)
                            em = dp.tile([DH, 512], f32, tag="em")
                            nc.vector.tensor_scalar_min(em[:, :], ph[:, :], 0.0)
                            nc.scalar.activation(em[:, :], em[:, :], AF.Exp)
                            px2 = dp.tile([DH, 512], f32, tag="px2")
                            nc.vector.tensor_scalar_max(px2[:, :], ph[:, :], 0.0)
                            e1 = dp.tile([DH, 512], bf16, tag="e1")
                            nc.vector.scalar_tensor_tensor(
                                e1[:, :], em[:, :], -1.0, px2[:, :],
                                OP.add, OP.add)
                            po_ = ps1.tile([1, 512], f32, tag="po")
                            nc.tensor.matmul(po_[:, :], w2t[:, :], e1[:, :],
                                             start=True, stop=True)
                            nc.vector.tensor_copy(orow[:, q * 512:(q + 1) * 512],
                                                  po_[:, :])
                        nc.sync.dma_start(out=pout[:, e0:e0 + DEC_CH],
                                          in_=orow[:, :])

    nc.compile()
    return nc


def kernel(features, edge_index, edges, T_f_batch, T_cf_batch, params):
    from concourse.bass_utils import run_bass_kernel_spmd

    features = np.asarray(features, np.float32)
    edge_index = np.asarray(edge_index)
    edges = np.asarray(edges)
    T_f = np.asarray(T_f_batch, np.float32)
    T_cf = np.asarray(T_cf_batch, np.float32)
    params = {k: np.asarray(v, np.float32) for k, v in params.items()}

    jw = params["jk_w"]
    wjk = np.exp(jw - jw.max())
    wjk = wjk / wjk.sum()

    kh = hash((edge_index.tobytes(), edges.tobytes(), wjk.tobytes()))
    if kh in _CACHE:
        host, nc = _CACHE[kh]
    else:
        host = _prep(edge_index, edges)
        host["wjk"] = wjk.astype(np.float64)
        nc = _build(host)
        _CACHE[kh] = (host, nc)

    oon = host["old_of_new"]
    featp = np.zeros((NPAD, DF), np.float32)
    mreal = oon >= 0
    featp[np.nonzero(mreal)[0]] = features[oon[mreal]]
    shared = {"featT": np.ascontiguousarray(featp.T).astype(BF16)}
    for l in range(3):
        wl = params[f"Wl{l}"] * params[f"al{l}"][None, :]
        wr = params[f"Wr{l}"]
        wc = np.zeros((wl.shape[0], ROW), np.float32)
        wc[:, 0:DH] = wl
        wc[:, 128:128 + DH] = wr
        shared[f"wcat{l}"] = wc.astype(BF16)
        shared[f"arv{l}"] = params[f"ar{l}"][None, :].astype(np.float32)
        shared[f"gam{l}"] = params[f"gamma{l}"][None, :].astype(np.float32)
        shared[f"bet{l}"] = params[f"beta{l}"][None, :].astype(np.float32)
    shared["w1l"] = params["W1"].astype(BF16)
    shared["w2"] = params["W2"].astype(BF16)

    in_maps = []
    for c in range(NCORES):
        m = dict(shared)
        for p in range(2):
            m[f"idxm{p}"] = host["idx_main"][p][c]
        base = c * NPC
        hi_nodes = host["cores"][c][1]["nodes"]
        hi_pos = np.empty(NPC, np.int64)
        hi_pos[hi_nodes] = np.arange(NPC)
        m["idxhi"] = _wrap16(hi_pos, NPC)
        own = base + np.arange(NPC)
        m["idxar0"] = _wrap16(np.where(own < HALF, own, HALF), NPC)
        m["idxar1"] = _wrap16(np.where(own >= HALF, own - HALF, HALF), NPC)
        ho = base + hi_nodes
        m["idxar2"] = _wrap16(np.where(ho < HALF, ho, HALF), NPC)
        m["idxar3"] = _wrap16(np.where(ho >= HALF, ho - HALF, HALF), NPC)
        ei = host["ei"][c * ELC:(c + 1) * ELC]
        ej = host["ej"][c * ELC:(c + 1) * ELC]
        m["idxd0"] = _wrap16(np.where(ei < HALF, ei, HALF), ELC)
        m["idxd1"] = _wrap16(np.where(ei >= HALF, ei - HALF, HALF), ELC)
        m["idxd2"] = _wrap16(np.where(ej < HALF, ej, HALF), ELC)
        m["idxd3"] = _wrap16(np.where(ej >= HALF, ej - HALF, HALF), ELC)
        m["tf"] = np.ascontiguousarray(T_f[c * ELC:(c + 1) * ELC][None, :])
        m["tcf"] = np.ascontiguousarray(T_cf[c * ELC:(c + 1) * ELC][None, :])
        in_maps.append(m)

    res = run_bass_kernel_spmd(nc, in_maps, core_ids=list(range(NCORES)))

    zp = np.concatenate([res.results[c]["z_out"] for c in range(NCORES)], 0)
    z = np.empty((N, DZ), np.float32)
    z[oon[mreal]] = zp[mreal]
    pf = np.concatenate([res.results[c]["pf"][0] for c in range(NCORES)])
    pcf = np.concatenate([res.results[c]["pcf"][0] for c in range(NCORES)])
    return z, pf, pcf
